# revision 21
# baseline (speedup 1.0000x reference)
"""Trainium2 Bass kernel for nn_CustomLlamaModel (2-layer MQA llama, B=1 S=2048
H=1024 HQ=16 HKV=1 FF=4096), data-parallel over 8 NeuronCores.

Strategy: token-sharded data parallelism (256 tokens/core) with fully
replicated weights.  Each core computes q (all 16 heads), k, v for its own
tokens; k is roped locally and k||v is AllGathered (64KB in -> 512KB out, the
ONLY collective per layer).  Attention (full softmax over all 2048 keys, MQA),
o-proj, and the whole MLP are then core-local -- no ReduceScatter, no
activation AllGather.  Residual stream is feature-major [128, 8*256] fp32 in
SBUF.  Matmuls run bf16 with fp32 PSUM; softmax denominator rides the attn@v
matmul as a ones-column in v_tok; rmsnorm uses exp(-0.5*ln(ms)) so the scalar
engine never leaves the Exp/Ln activation table between norms and softmax.
ln1/ln2 and 1/sqrt(D) are folded into weights host-side; embedding gather is
host-side numpy.  MLP weights (24MB/layer) are streamed through SBUF tile
rings, prefetched during attention.
"""
import sys

sys.path.insert(0, "/opt/trn_rl_repo")

import ml_dtypes
import numpy as np
import orjson

import concourse.bass as bass
import concourse.mybir as mybir
import concourse.tile as tile
from concourse import bass_utils
from concourse.masks import make_identity

# ---------------------------------------------------------------------------
# Walrus in this container supports only ONE sync-wait per instruction, but
# Tile's scheduler emits multi-wait instructions.  Post-process the BIR JSON:
# split each multi-wait instruction into single-wait NoOps (same engine,
# program-order before the original).
# ---------------------------------------------------------------------------
_orig_to_json_bytes = bass.Bass.to_json_bytes
_MW = [0]


def _split_multiwait(d):
    changed = False

    def fix_block(bb):
        nonlocal changed
        insts = bb.get("instructions")
        if not insts:
            return
        out = []
        for ins in insts:
            si = ins.get("sync_info")
            if si:
                ow = si.get("on_wait") or []
                if len(ow) > 1:
                    changed = True
                    for w in ow[:-1]:
                        _MW[0] += 1
                        out.append({
                            "debug": ins.get("debug", 0),
                            "engine": ins["engine"],
                            "ins": [],
                            "outs": [],
                            "name": f"{ins['name']}-mw{_MW[0]}",
                            "opcode": "NoOp",
                            "sync_info": {"on_update": [], "on_wait": [w]},
                        })
                    si["on_wait"] = [ow[-1]]
            out.append(ins)
        bb["instructions"] = out

    def rec(o):
        if isinstance(o, dict):
            if isinstance(o.get("instructions"), list):
                fix_block(o)
            for v in o.values():
                rec(v)
        elif isinstance(o, list):
            for v in o:
                rec(v)

    for fn in d.get("functions", []):
        rec(fn)
    return changed


def _patched_to_json_bytes(self):
    raw = _orig_to_json_bytes(self)
    d = orjson.loads(raw)
    if _split_multiwait(d):
        return orjson.dumps(d)
    return raw


bass.Bass.to_json_bytes = _patched_to_json_bytes

# ---------------------------------------------------------------------------
# Model / sharding constants
# ---------------------------------------------------------------------------
S, H, D, HQ, FF, L, V = 2048, 1024, 64, 16, 4096, 2, 32000
EPS = 1e-6
NCORES = 8
TOK = S // NCORES       # tokens per core (256)
HT = H // 128           # hidden feature tiles (8)
FT = FF // 128          # ff tiles (32)
KT = S // 128           # key-token tiles (16)
HP = HQ // 2            # head pairs (8)
F32 = mybir.dt.float32
F32R = mybir.dt.float32r
BF16 = mybir.dt.bfloat16
FP8 = mybir.dt.float8e4
DR = mybir.MatmulPerfMode.DoubleRow
WS = 64.0                # fp8 weight scale (wg, wd)
WSU = 16.0               # fp8 wu scale (act = WSU*silu*u must stay < 448)
WS2 = WS * WSU
MUL = mybir.AluOpType.mult
ADD = mybir.AluOpType.add
AF = mybir.ActivationFunctionType
RG = [list(range(NCORES))]
BF = ml_dtypes.bfloat16

_CACHED_NC = None
DEBUG = False


def _build_nc():
    nc = bass.Bass()
    x0 = nc.dram_tensor("x0", [128, HT * TOK], F32, kind="ExternalInput")
    cosq = nc.dram_tensor("cosq", [64, 512], BF16, kind="ExternalInput")
    sinq = nc.dram_tensor("sinq", [64, 512], BF16, kind="ExternalInput")
    cosa = nc.dram_tensor("cosa", [64, S], BF16, kind="ExternalInput")
    sina = nc.dram_tensor("sina", [64, S], BF16, kind="ExternalInput")
    h1a = nc.dram_tensor("h1a", [128, 4 * HT * 512], BF16,
                         kind="ExternalInput")
    h1loc = nc.dram_tensor("h1loc", [128, HT * TOK], BF16,
                           kind="ExternalInput")
    W = []
    for l in range(L):
        W.append({
            "wq": nc.dram_tensor(f"wq{l}", [128, HT * H], BF16,
                                 kind="ExternalInput"),
            "wkv": nc.dram_tensor(f"wkv{l}", [128, HT * 128], BF16,
                                  kind="ExternalInput"),
            "wo": nc.dram_tensor(f"wo{l}", [128, HT * H], BF16,
                                 kind="ExternalInput"),
            "wg": nc.dram_tensor(f"wg{l}", [128, FT * H], BF16,
                                 kind="ExternalInput"),
            "wu": nc.dram_tensor(f"wu{l}", [128, FT * H], BF16,
                                 kind="ExternalInput"),
            "wd": nc.dram_tensor(f"wd{l}", [128, FT * H], BF16,
                                 kind="ExternalInput"),
        })
    xout = nc.dram_tensor("xout", [128, HT * TOK], F32, kind="ExternalOutput")
    dbg = {}
    if DEBUG:
        dbg["h1"] = nc.dram_tensor("dbg_h1", [128, HT * TOK], BF16,
                                   kind="ExternalOutput")
        dbg["qraw"] = nc.dram_tensor("dbg_qraw", [64, HQ * TOK], BF16,
                                     kind="ExternalOutput")
        dbg["kvb"] = nc.dram_tensor("dbg_kvb", [128, TOK], BF16,
                                    kind="ExternalOutput")
        dbg["kT"] = nc.dram_tensor("dbg_kT", [64, S], BF16,
                                   kind="ExternalOutput")
        dbg["vtok"] = nc.dram_tensor("dbg_vtok", [128, KT * 65], BF16,
                                     kind="ExternalOutput")
        dbg["q2"] = nc.dram_tensor("dbg_q2", [64, 512], BF16,
                                   kind="ExternalOutput")
        dbg["oT"] = nc.dram_tensor("dbg_oT", [128, HP * TOK], BF16,
                                   kind="ExternalOutput")
        dbg["xatt"] = nc.dram_tensor("dbg_xatt", [128, HT * TOK], F32,
                                     kind="ExternalOutput")
        dbg["act"] = nc.dram_tensor("dbg_act", [128, FT * TOK], BF16,
                                    kind="ExternalOutput")
        dbg["xl0"] = nc.dram_tensor("dbg_xl0", [128, HT * TOK], F32,
                                    kind="ExternalOutput")

    with tile.TileContext(nc) as tc:
        with (
            tc.tile_pool(name="const", bufs=1) as pconst,
            tc.tile_pool(name="resid", bufs=1) as presid,
            tc.tile_pool(name="hbuf", bufs=1) as ph,
            tc.tile_pool(name="qraw", bufs=1) as pq,
            tc.tile_pool(name="kvt", bufs=1) as pkv,
            tc.tile_pool(name="obuf", bufs=1) as poT,
            tc.tile_pool(name="act", bufs=1) as pact,
            tc.tile_pool(name="wbig", bufs=1) as pw,
            tc.tile_pool(name="wstream", bufs=6) as pws,
            tc.tile_pool(name="small", bufs=3) as psmall,
            tc.tile_pool(name="exp", bufs=3) as pexp,
            tc.tile_pool(name="x2p", bufs=1) as px2,
            tc.tile_pool(name="dram", bufs=2, space="DRAM") as pdram,
        ):
            # ---------------- constants ----------------
            identf = pconst.tile([128, 128], F32, tag="identf")
            make_identity(nc, identf[:])
            ident = pconst.tile([128, 128], BF16, tag="ident")
            nc.vector.tensor_copy(ident[:], identf[:])
            onesf = pconst.tile([128, 128], F32, tag="onesf")
            nc.vector.memset(onesf[:], 1.0)
            onesr = pconst.tile([128, 128], F32R, tag="onesr")
            nc.vector.tensor_copy(onesr[:], onesf[:])
            epst = pconst.tile([128, 1], F32, tag="eps")
            nc.gpsimd.memset(epst[:], EPS)
            cosq_sb = pconst.tile([64, 512], BF16, tag="cosq")
            sinq_sb = pconst.tile([64, 512], BF16, tag="sinq")
            nc.sync.dma_start(cosq_sb[:], cosq[:])
            nc.sync.dma_start(sinq_sb[:], sinq[:])
            cosa_sb = pconst.tile([64, S], BF16, tag="cosa")
            sina_sb = pconst.tile([64, S], BF16, tag="sina")
            nc.sync.dma_start(cosa_sb[:], cosa[:])
            nc.sync.dma_start(sina_sb[:], sina[:])
            h1l_sb = pconst.tile([128, HT * TOK], BF16, tag="h1l")
            nc.sync.dma_start(h1l_sb[:], h1loc[:])

            # warmup: absorb the collective setup / barrier cost early
            wrm_i = pdram.tile([128, 16], BF16, tag="warm_i")
            wrm_o = pdram.tile([NCORES * 128, 16], BF16, tag="warm_o",
                               addr_space="Shared")
            nc.gpsimd.collective_compute(
                "AllGather", mybir.AluOpType.bypass, replica_groups=RG,
                ins=[wrm_i[:].opt()], outs=[wrm_o[:].opt()],
            )

            # residual x, feature-major: block ht -> cols [ht*TOK:(ht+1)*TOK]
            # (loaded after the layer-0 critical-path DMAs, below)
            x_sb = presid.tile([128, HT * TOK], F32, tag="x")

            def rmsnorm(tag, dt=BF16):
                """x_sb -> h (same layout).  rsqrt via exp(-.5*ln(ms))
                to stay on the Exp/Ln activation table."""
                h_sb = ph.tile([128, HT * TOK], dt, tag="h" + str(dt))
                x2 = px2.tile([128, HT * TOK], F32R, tag="x2")
                with tc.tile_pool(name=f"ps_n_{tag}", bufs=1,
                                  space="PSUM") as pps:
                    ssq = pps.tile([1, TOK], F32, tag="ssq")
                    for ht in range(HT):
                        hs = slice(ht * TOK, (ht + 1) * TOK)
                        nc.vector.tensor_tensor(x2[:, hs], x_sb[:, hs],
                                                x_sb[:, hs], op=MUL)
                        nc.tensor.matmul(
                            ssq[:], onesr[:, 0:1], x2[:, hs],
                            start=(ht == 0), stop=(ht == HT - 1),
                        )
                    lnv = psmall.tile([1, TOK], F32, tag="lnv")
                    nc.scalar.activation(lnv[:], ssq[:], AF.Ln,
                                         bias=epst[0:1, :], scale=1.0 / H)
                    rinv = psmall.tile([1, TOK], F32R, tag="rinv")
                    nc.scalar.activation(rinv[:], lnv[:], AF.Exp, scale=-0.5)
                    rb = pps.tile([128, TOK], F32, tag="rb")
                    nc.tensor.matmul(rb[:], onesr[0:1, :], rinv[:],
                                     start=True, stop=True)
                    for ht in range(HT):
                        hs = slice(ht * TOK, (ht + 1) * TOK)
                        nc.vector.tensor_tensor(h_sb[:, hs], x_sb[:, hs],
                                                rb[:], op=MUL)
                return h_sb

            for l in range(L):
                w = W[l]
                # ---------------- weight loads ----------------
                # layer 0: kv-path DMAs first (critical path at startup);
                # wq/wo/x0 follow in the queue.  later layers: all up front.
                wq_sb = pw.tile([128, HT * H], BF16, tag="wq")
                wkv_sb = pw.tile([128, HT * 128], BF16, tag="wkv")
                wo_sb = pw.tile([128, HT * H], BF16, tag="wo")
                nc.sync.dma_start(wkv_sb[:], w["wkv"][:])
                if l > 0:
                    nc.sync.dma_start(wq_sb[:], w["wq"][:])
                    nc.sync.dma_start(wo_sb[:], w["wo"][:])

                if l == 0:
                    h1 = h1l_sb
                else:
                    h1 = rmsnorm(f"a{l}")

                with tc.tile_pool(name=f"ps_qkv{l}", bufs=1,
                                  space="PSUM") as pps:
                    kT = pkv.tile([128, S], BF16, tag="kT")
                    nc.vector.memset(kT[64:128, :], 0.0)
                    vT = pkv.tile([64, S], BF16, tag="vT")
                    if l == 0:
                        # layer 0: host-normalized h1 for ALL tokens is an
                        # input; compute k,v for all 2048 tokens locally
                        # (redundant per core) -- no collective needed, so
                        # the CC barrier overlaps real work.
                        for c4 in range(4):
                            h1c = pws.tile([128, HT * 512], BF16, tag="h1c",
                                           bufs=2)
                            nc.sync.dma_start(
                                h1c[:], h1a[:, c4 * HT * 512:
                                            (c4 + 1) * HT * 512])
                            pkv_ps = pps.tile([128, 512], F32, tag="pkv",
                                              bufs=2)
                            for k in range(HT):
                                nc.tensor.matmul(
                                    pkv_ps[:],
                                    wkv_sb[:, k * 128:(k + 1) * 128],
                                    h1c[:, k * 512:(k + 1) * 512],
                                    start=(k == 0), stop=(k == HT - 1),
                                )
                            cs4 = slice(c4 * 512, (c4 + 1) * 512)
                            rotk = psmall.tile([64, 512], BF16, tag="rotk")
                            nc.vector.tensor_copy(rotk[0:32, :],
                                                  pkv_ps[32:64, :])
                            nc.vector.tensor_copy(rotk[32:64, :],
                                                  pkv_ps[0:32, :])
                            nc.vector.tensor_tensor(
                                rotk[:], rotk[:], sina_sb[:, cs4], op=MUL)
                            nc.vector.tensor_tensor(
                                kT[0:64, cs4], pkv_ps[0:64, :],
                                cosa_sb[:, cs4], op=MUL)
                            nc.vector.tensor_tensor(
                                kT[0:64, cs4], kT[0:64, cs4], rotk[:],
                                op=ADD)
                            nc.vector.tensor_copy(vT[:, cs4],
                                                  pkv_ps[64:128, :])
                    else:
                        # local kv + rope + AllGather
                        kvb = psmall.tile([128, TOK], BF16, tag="kvb")
                        pkv_ps = pps.tile([128, 512], F32, tag="pkv",
                                          bufs=2)
                        for k in range(HT):
                            nc.tensor.matmul(
                                pkv_ps[:, 0:TOK],
                                wkv_sb[:, k * 128:(k + 1) * 128],
                                h1[:, k * TOK:(k + 1) * TOK],
                                start=(k == 0), stop=(k == HT - 1),
                            )
                        rotk = psmall.tile([64, 512], BF16, tag="rotk")
                        nc.vector.tensor_copy(rotk[0:32, 0:TOK],
                                              pkv_ps[32:64, 0:TOK])
                        nc.vector.tensor_copy(rotk[32:64, 0:TOK],
                                              pkv_ps[0:32, 0:TOK])
                        nc.vector.tensor_tensor(rotk[:, 0:TOK],
                                                rotk[:, 0:TOK],
                                                sinq_sb[:, 0:TOK], op=MUL)
                        nc.vector.tensor_tensor(kvb[0:64, :],
                                                pkv_ps[0:64, 0:TOK],
                                                cosq_sb[:, 0:TOK], op=MUL)
                        nc.vector.tensor_tensor(kvb[0:64, :], kvb[0:64, :],
                                                rotk[:, 0:TOK], op=ADD)
                        nc.vector.tensor_copy(kvb[64:128, :],
                                              pkv_ps[64:128, 0:TOK])

                        kv_in = pdram.tile([128, TOK], BF16, tag="kv_in")
                        nc.sync.dma_start(kv_in[:], kvb[:])
                        kv_out = pdram.tile([NCORES * 128, TOK], BF16,
                                            tag="kv_out",
                                            addr_space="Shared")
                        nc.gpsimd.collective_compute(
                            "AllGather", mybir.AluOpType.bypass,
                            replica_groups=RG,
                            ins=[kv_in[:].opt()], outs=[kv_out[:].opt()],
                        )

                    # ---------------- q projection ----------------
                    if l == 0:
                        nc.sync.dma_start(wq_sb[:], w["wq"][:])
                        nc.sync.dma_start(wo_sb[:], w["wo"][:])
                        nc.sync.dma_start(x_sb[:], x0[:])
                    # q_raw head-blocked [64, 16*256]: head h at cols h*TOK
                    q_raw = pq.tile([64, HQ * TOK], BF16, tag="q_raw")
                    for mp in range(4):      # m-tile pairs -> [128,512] psum
                        pq_ps = pps.tile([128, 2 * TOK], F32, tag="pq",
                                         bufs=3)
                        for j in range(2):
                            m = 2 * mp + j
                            for k in range(HT):
                                nc.tensor.matmul(
                                    pq_ps[:, j * TOK:(j + 1) * TOK],
                                    wq_sb[:, k * H + m * 128:
                                          k * H + (m + 1) * 128],
                                    h1[:, k * TOK:(k + 1) * TOK],
                                    start=(k == 0), stop=(k == HT - 1),
                                    skip_group_check=True,
                                )
                        for j in range(2):
                            m = 2 * mp + j
                            js = slice(j * TOK, (j + 1) * TOK)
                            nc.vector.tensor_copy(
                                q_raw[:, (2 * m) * TOK:(2 * m + 1) * TOK],
                                pq_ps[0:64, js])
                            nc.vector.tensor_copy(
                                q_raw[:, (2 * m + 1) * TOK:
                                      (2 * m + 2) * TOK],
                                pq_ps[64:128, js])

                    # ---------------- gather k/v, build v_tok ----------
                    if l > 0:
                        for s in range(NCORES):
                            nc.sync.dma_start(
                                kT[0:64, s * TOK:(s + 1) * TOK],
                                kv_out[s * 128:s * 128 + 64, :])
                            nc.sync.dma_start(
                                vT[:, s * TOK:(s + 1) * TOK],
                                kv_out[s * 128 + 64:(s + 1) * 128, :])
                    v_tok = pkv.tile([128, KT * 65], BF16, tag="v_tok")
                    nc.vector.memset(v_tok[:], 1.0)
                    for kt in range(KT):
                        pvt = pps.tile([128, 64], BF16, tag="pvt", bufs=2)
                        nc.tensor.transpose(
                            pvt[:], vT[:, kt * 128:(kt + 1) * 128],
                            ident[0:64, 0:64])
                        nc.vector.tensor_copy(
                            v_tok[:, kt * 65:kt * 65 + 64], pvt[:])

                if DEBUG and l == 0:
                    nc.sync.dma_start(dbg["qraw"][:], q_raw[:])
                    nc.sync.dma_start(dbg["kT"][:], kT[0:64, :])
                    nc.sync.dma_start(dbg["vtok"][:], v_tok[:])
                # ---------------- rope q (all head pairs up front) --------
                q2s = []
                for hp in range(HP):
                    cs = slice(hp * 512, (hp + 1) * 512)
                    q2 = psmall.tile([128, 512], BF16, tag="q2", bufs=8)
                    nc.vector.memset(q2[64:128, :], 0.0)
                    rot = psmall.tile([64, 512], BF16, tag="rotq", bufs=2)
                    nc.vector.tensor_copy(rot[0:32, :], q_raw[32:64, cs])
                    nc.vector.tensor_copy(rot[32:64, :], q_raw[0:32, cs])
                    nc.vector.tensor_tensor(rot[:], rot[:], sinq_sb[:],
                                            op=MUL)
                    nc.vector.tensor_tensor(q2[0:64, :], q_raw[:, cs],
                                            cosq_sb[:], op=MUL)
                    nc.vector.tensor_tensor(q2[0:64, :], q2[0:64, :],
                                            rot[:], op=ADD)
                    q2s.append(q2)
                if DEBUG and l == 0:
                    nc.sync.dma_start(dbg["q2"][:], q2s[0][0:64, :])

                # ---------------- attention ----------------
                # oT [128, 8*256]: block hp holds heads 2hp (rows 0:64) and
                # 2hp+1 (rows 64:128) for the core's 256 tokens.
                oT = poT.tile([128, HP * TOK], BF16, tag="oT")
                with tc.tile_pool(name=f"ps_att{l}", bufs=1,
                                  space="PSUM") as ppa:
                    # Software-pipelined: av runs 2 ktp slots behind scores
                    # (exp always complete -> PE never stalls mid-pair), and
                    # each pair's finalize (recip/broadcast/normalize) is
                    # interleaved into the NEXT pair's score stream so the
                    # PE never head-of-line blocks at a pair boundary.
                    pavs = {}
                    pend = []       # [(hp, ktp, et, last)] awaiting av
                    fin_pend = []   # [(hp, rec)] awaiting broadcast+norm

                    def emit_av(hp, ktp, et, last):
                        pav = pavs[hp]
                        for j in range(2):
                            kt = 2 * ktp + j
                            nc.tensor.matmul(
                                pav[:], v_tok[:, kt * 65:(kt + 1) * 65],
                                et[:, j * 512:(j + 1) * 512],
                                start=(ktp == 0 and j == 0),
                                stop=(last and j == 1),
                                skip_group_check=True,
                            )
                        if last:
                            rec = psmall.tile([1, 512], F32R, tag="rec",
                                              bufs=2)
                            with nc.allow_low_precision(reason="f32r"):
                                nc.vector.reciprocal(rec[:], pav[64:65, :])
                            fin_pend.append((hp, rec))

                    def emit_fin(hp, rec):
                        pav = pavs.pop(hp)
                        prb = ppa.tile([128, 1024], F32, tag="psc", bufs=3)
                        nc.tensor.matmul(prb[0:64, 0:512],
                                         onesr[0:1, 0:64], rec[:],
                                         start=True, stop=True)
                        rbs = psmall.tile([64, 512], F32, tag="rbs", bufs=2)
                        nc.vector.tensor_copy(rbs[:], prb[0:64, 0:512])
                        nc.vector.tensor_tensor(
                            oT[0:64, hp * TOK:(hp + 1) * TOK],
                            pav[0:64, 0:TOK], rbs[:, 0:TOK], op=MUL)
                        nc.vector.tensor_tensor(
                            oT[64:128, hp * TOK:(hp + 1) * TOK],
                            pav[0:64, TOK:512], rbs[:, TOK:512], op=MUL)

                    for hp in range(HP):
                        q2 = q2s[hp]
                        pav = ppa.tile([65, 512], F32, tag="pav", bufs=2)
                        pavs[hp] = pav
                        for ktp in range(8):
                            psc = ppa.tile([128, 1024], F32, tag="psc",
                                           bufs=3)
                            for j in range(2):
                                kt = 2 * ktp + j
                                nc.tensor.matmul(
                                    psc[:, j * 512:(j + 1) * 512],
                                    kT[:, kt * 128:(kt + 1) * 128],
                                    q2[:], start=True, stop=True,
                                    skip_group_check=True,
                                )
                            et = pexp.tile([128, 1024], BF16, tag="et",
                                           bufs=5)
                            nc.scalar.activation(et[:], psc[:], AF.Exp)
                            if len(pend) >= 3:
                                emit_av(*pend.pop(0))
                            if fin_pend and ktp == 4:
                                emit_fin(*fin_pend.pop(0))
                            pend.append((hp, ktp, et, ktp == 7))
                    for job in pend:
                        emit_av(*job)
                    for hp_f, rec_f in fin_pend:
                        emit_fin(hp_f, rec_f)

                # ---------------- o-proj + residual ----------------
                with tc.tile_pool(name=f"ps_o{l}", bufs=1,
                                  space="PSUM") as ppo:
                    for mp in range(4):
                        po = ppo.tile([128, 2 * TOK], F32, tag="po", bufs=2)
                        for j in range(2):
                            m = 2 * mp + j
                            for hp in range(HP):
                                nc.tensor.matmul(
                                    po[:, j * TOK:(j + 1) * TOK],
                                    wo_sb[:, hp * H + m * 128:
                                          hp * H + (m + 1) * 128],
                                    oT[:, hp * TOK:(hp + 1) * TOK],
                                    start=(hp == 0), stop=(hp == HT - 1),
                                    skip_group_check=True,
                                )
                        for j in range(2):
                            m = 2 * mp + j
                            ms = slice(m * TOK, (m + 1) * TOK)
                            nc.vector.tensor_tensor(
                                x_sb[:, ms], x_sb[:, ms],
                                po[:, j * TOK:(j + 1) * TOK], op=ADD)

                if DEBUG and l == 0:
                    nc.sync.dma_start(dbg["oT"][:], oT[:])
                    nc.sync.dma_start(dbg["xatt"][:], x_sb[:])
                # ---------------- mlp ----------------
                h2 = rmsnorm(f"m{l}")
                act = pact.tile([128, FT * TOK], BF16, tag="act")
                with tc.tile_pool(name=f"ps_mlp{l}", bufs=1,
                                  space="PSUM") as ppm:
                    for f in range(FT):
                        wgf = pws.tile([128, H], BF16, tag="wgf")
                        wuf = pws.tile([128, H], BF16, tag="wuf")
                        nc.sync.dma_start(wgf[:],
                                          w["wg"][:, f * H:(f + 1) * H])
                        nc.gpsimd.dma_start(wuf[:],
                                            w["wu"][:, f * H:(f + 1) * H])
                        # NOTE: start=True marks the whole 2KB PSUM bank
                        # pending-zero, so the g and u accumulation groups
                        # sharing this tile must run sequentially, not
                        # interleaved.
                        pgu = ppm.tile([128, 2 * TOK], F32, tag="pgu",
                                       bufs=3)
                        for k in range(HT):
                            nc.tensor.matmul(
                                pgu[:, 0:TOK],
                                wgf[:, k * 128:(k + 1) * 128],
                                h2[:, k * TOK:(k + 1) * TOK],
                                start=(k == 0), stop=(k == HT - 1),
                                skip_group_check=True,
                            )
                        for k in range(HT):
                            nc.tensor.matmul(
                                pgu[:, TOK:2 * TOK],
                                wuf[:, k * 128:(k + 1) * 128],
                                h2[:, k * TOK:(k + 1) * TOK],
                                start=(k == 0), stop=(k == HT - 1),
                                skip_group_check=True,
                            )
                        fs = slice(f * TOK, (f + 1) * TOK)
                        asl = psmall.tile([128, TOK], BF16, tag="asl")
                        nc.scalar.activation(asl[:], pgu[:, 0:TOK], AF.Silu)
                        nc.vector.tensor_tensor(act[:, fs], asl[:],
                                                pgu[:, TOK:2 * TOK], op=MUL)

                    # down: m outer (one sequential accumulation group per
                    # psum tile), fk inner; wd streamed in per-m 1MB chunks
                    for m in range(HT):
                        wdm = pws.tile([128, FT * 128], BF16, tag="wdm",
                                       bufs=2)
                        nc.gpsimd.dma_start(
                            wdm[:], w["wd"][:, m * FT * 128:
                                            (m + 1) * FT * 128])
                        pd = ppm.tile([128, TOK], F32, tag="pd", bufs=3)
                        for fk in range(FT):
                            nc.tensor.matmul(
                                pd[:], wdm[:, fk * 128:(fk + 1) * 128],
                                act[:, fk * TOK:(fk + 1) * TOK],
                                start=(fk == 0), stop=(fk == FT - 1),
                            )
                        ms = slice(m * TOK, (m + 1) * TOK)
                        nc.vector.tensor_tensor(
                            x_sb[:, ms], x_sb[:, ms], pd[:], op=ADD)
                        if l == L - 1:
                            nc.sync.dma_start(xout[:, ms],
                                              x_sb[:, ms])
                if DEBUG and l == 0:
                    nc.sync.dma_start(dbg["act"][:], act[:])
                    nc.sync.dma_start(dbg["xl0"][:], x_sb[:])

    return nc


def _get_nc():
    global _CACHED_NC
    if _CACHED_NC is None:
        _CACHED_NC = _build_nc()
    return _CACHED_NC


def _pack_km(wT, K, M):
    """[K*128, M*128] (contract-major) -> [128, K*M*128], col k*M*128+m*128+c
    = tile (k, m) so tile slice [:, k*(M*128)+m*128 : +128] is lhsT."""
    return np.ascontiguousarray(
        wT.reshape(K, 128, M, 128).transpose(1, 0, 2, 3).reshape(128, -1))


def _pack_fk(wT, K, Fn):
    """[K*128, Fn*128] -> [128, Fn*K*128], col f*(K*128)+k*128+c = tile
    (k, f): f-chunk [:, f*K*128 : (f+1)*K*128] holds all K contract tiles."""
    return np.ascontiguousarray(
        wT.reshape(K, 128, Fn, 128).transpose(1, 2, 0, 3).reshape(128, -1))


def _host_prep(inputs):
    """Fold ln/scale into weights, pre-pack DMA-friendly layouts, embed
    gather, per-core rope tables.  Returns in_maps (list of dicts)."""
    ids = np.asarray(inputs["input_ids"])[0]          # [S] int32
    embed = np.asarray(inputs["embed"], np.float32)   # [V, H]
    x = embed[ids]                                    # [S, H]
    ms = (x * x).mean(-1, keepdims=True) + EPS
    h1_full = (x / np.sqrt(ms)).astype(np.float32)    # pre-ln-fold norm

    inv = 1.0 / (10000.0 ** (np.arange(0, D, 2, dtype=np.float32) / D))
    scale = np.float32(1.0 / np.sqrt(D))

    def bf(a):
        return np.ascontiguousarray(a).astype(BF)

    # shared (per-layer) weights, packed once
    shared = {}
    for l in range(L):
        ln1 = np.asarray(inputs["ln1"], np.float32)[l]
        ln2 = np.asarray(inputs["ln2"], np.float32)[l]
        wq = np.asarray(inputs["Wq"], np.float32)[l] * ln1[None, :] * scale
        wk = np.asarray(inputs["Wk"], np.float32)[l] * ln1[None, :]
        wv = np.asarray(inputs["Wv"], np.float32)[l] * ln1[None, :]
        wo = np.asarray(inputs["Wo"], np.float32)[l]
        wg = np.asarray(inputs["Wg"], np.float32)[l] * ln2[None, :]
        wu = np.asarray(inputs["Wu"], np.float32)[l] * ln2[None, :]
        wd = np.asarray(inputs["Wd"], np.float32)[l]
        wkv = np.concatenate([wk, wv], axis=0)        # [128, H]
        shared[f"wq{l}"] = bf(_pack_km(wq.T, HT, HT))
        shared[f"wkv{l}"] = bf(_pack_km(wkv.T, HT, 1))
        shared[f"wo{l}"] = bf(_pack_km(wo.T.reshape(H, H), HT, HT))
        shared[f"wg{l}"] = bf(_pack_fk(wg.T, HT, FT))
        shared[f"wu{l}"] = bf(_pack_fk(wu.T, HT, FT))
        shared[f"wd{l}"] = bf(_pack_fk(wd.T, FT, HT))

    # global rope tables + packed h1_all (shared across cores)
    frA = np.arange(S, dtype=np.float32)[:, None] * inv[None, :]
    cosTA = np.cos(frA).T.astype(np.float32)
    sinTA = np.sin(frA).T.astype(np.float32)
    cosA = np.tile(cosTA, (2, 1))                     # [64, S]
    sinA = np.concatenate([-sinTA, sinTA], 0)
    h1a = np.ascontiguousarray(
        h1_full.T.reshape(HT, 128, 4, 512).transpose(1, 2, 0, 3)
        .reshape(128, -1))
    shared["cosa"] = bf(cosA)
    shared["sina"] = bf(sinA)
    shared["h1a"] = bf(h1a)

    in_maps = []
    for c in range(NCORES):
        pos = np.arange(c * TOK, (c + 1) * TOK, dtype=np.float32)
        freqs = pos[:, None] * inv[None, :]           # [TOK, 32]
        cosT = np.cos(freqs).T.astype(np.float32)     # [32, TOK]
        sinT = np.sin(freqs).T.astype(np.float32)
        cosQ = np.tile(np.tile(cosT, (2, 1)), (1, 2))             # [64, 512]
        sinQ = np.tile(np.concatenate([-sinT, sinT], 0), (1, 2))  # [64, 512]
        xT = x[c * TOK:(c + 1) * TOK, :].T            # [H, TOK]
        x0p = np.ascontiguousarray(
            xT.reshape(HT, 128, TOK).transpose(1, 0, 2).reshape(128, -1))
        h1loc = np.ascontiguousarray(
            h1_full[c * TOK:(c + 1) * TOK].T.reshape(HT, 128, TOK)
            .transpose(1, 0, 2).reshape(128, -1))
        m = {"x0": x0p, "cosq": bf(cosQ), "sinq": bf(sinQ),
             "h1loc": bf(h1loc)}
        m.update(shared)
        in_maps.append(m)
    return in_maps


def kernel(**inputs) -> np.ndarray:
    nc = _get_nc()
    in_maps = _host_prep(inputs)
    res = bass_utils.run_bass_kernel_spmd(
        nc, in_maps, core_ids=list(range(NCORES))
    )
    out = np.empty((1, S, H), np.float32)
    for c in range(NCORES):
        xp = res.results[c]["xout"]                   # [128, HT*TOK]
        xT = xp.reshape(128, HT, TOK).transpose(1, 0, 2).reshape(H, TOK)
        out[0, c * TOK:(c + 1) * TOK, :] = xT.T
    return out


# revision 22
# speedup vs baseline: 1.1911x; 1.1911x over previous
"""Trainium2 Bass kernel for nn_CustomLlamaModel (2-layer MQA llama, B=1 S=2048
H=1024 HQ=16 HKV=1 FF=4096), data-parallel over 8 NeuronCores.

Strategy: token-sharded data parallelism (256 tokens/core) with fully
replicated weights.  Each core computes q (all 16 heads), k, v for its own
tokens; k is roped locally and k||v is AllGathered (64KB in -> 512KB out, the
ONLY collective per layer).  Attention (full softmax over all 2048 keys, MQA),
o-proj, and the whole MLP are then core-local -- no ReduceScatter, no
activation AllGather.  Residual stream is feature-major [128, 8*256] fp32 in
SBUF.  Matmuls run bf16 with fp32 PSUM; softmax denominator rides the attn@v
matmul as a ones-column in v_tok; rmsnorm uses exp(-0.5*ln(ms)) so the scalar
engine never leaves the Exp/Ln activation table between norms and softmax.
ln1/ln2 and 1/sqrt(D) are folded into weights host-side; embedding gather is
host-side numpy.  MLP weights (24MB/layer) are streamed through SBUF tile
rings, prefetched during attention.
"""
import sys

sys.path.insert(0, "/opt/trn_rl_repo")

import ml_dtypes
import numpy as np
import orjson

import concourse.bass as bass
import concourse.mybir as mybir
import concourse.tile as tile
from concourse import bass_utils
from concourse.masks import make_identity

# ---------------------------------------------------------------------------
# Walrus in this container supports only ONE sync-wait per instruction, but
# Tile's scheduler emits multi-wait instructions.  Post-process the BIR JSON:
# split each multi-wait instruction into single-wait NoOps (same engine,
# program-order before the original).
# ---------------------------------------------------------------------------
_orig_to_json_bytes = bass.Bass.to_json_bytes
_MW = [0]


def _split_multiwait(d):
    changed = False

    def fix_block(bb):
        nonlocal changed
        insts = bb.get("instructions")
        if not insts:
            return
        out = []
        for ins in insts:
            si = ins.get("sync_info")
            if si:
                ow = si.get("on_wait") or []
                if len(ow) > 1:
                    changed = True
                    for w in ow[:-1]:
                        _MW[0] += 1
                        out.append({
                            "debug": ins.get("debug", 0),
                            "engine": ins["engine"],
                            "ins": [],
                            "outs": [],
                            "name": f"{ins['name']}-mw{_MW[0]}",
                            "opcode": "NoOp",
                            "sync_info": {"on_update": [], "on_wait": [w]},
                        })
                    si["on_wait"] = [ow[-1]]
            out.append(ins)
        bb["instructions"] = out

    def rec(o):
        if isinstance(o, dict):
            if isinstance(o.get("instructions"), list):
                fix_block(o)
            for v in o.values():
                rec(v)
        elif isinstance(o, list):
            for v in o:
                rec(v)

    for fn in d.get("functions", []):
        rec(fn)
    return changed


def _patched_to_json_bytes(self):
    raw = _orig_to_json_bytes(self)
    d = orjson.loads(raw)
    if _split_multiwait(d):
        return orjson.dumps(d)
    return raw


bass.Bass.to_json_bytes = _patched_to_json_bytes

# ---------------------------------------------------------------------------
# Model / sharding constants
# ---------------------------------------------------------------------------
S, H, D, HQ, FF, L, V = 2048, 1024, 64, 16, 4096, 2, 32000
EPS = 1e-6
NCORES = 8
TOK = S // NCORES       # tokens per core (256)
HT = H // 128           # hidden feature tiles (8)
FT = FF // 128          # ff tiles (32)
KT = S // 128           # key-token tiles (16)
HP = HQ // 2            # head pairs (8)
F32 = mybir.dt.float32
F32R = mybir.dt.float32r
BF16 = mybir.dt.bfloat16
FP8 = mybir.dt.float8e4
DR = mybir.MatmulPerfMode.DoubleRow
WS = 64.0                # fp8 weight scale (wg, wd)
WSU = 16.0               # fp8 wu scale (act = WSU*silu*u must stay < 448)
WS2 = WS * WSU
MUL = mybir.AluOpType.mult
ADD = mybir.AluOpType.add
AF = mybir.ActivationFunctionType
RG = [list(range(NCORES))]
BF = ml_dtypes.bfloat16

_CACHED_NC = None
DEBUG = False


def _build_nc():
    nc = bass.Bass()
    x0 = nc.dram_tensor("x0", [128, HT * TOK], F32, kind="ExternalInput")
    cosq = nc.dram_tensor("cosq", [64, 512], BF16, kind="ExternalInput")
    sinq = nc.dram_tensor("sinq", [64, 512], BF16, kind="ExternalInput")
    cosa = nc.dram_tensor("cosa", [64, S], BF16, kind="ExternalInput")
    sina = nc.dram_tensor("sina", [64, S], BF16, kind="ExternalInput")
    h1a = nc.dram_tensor("h1a", [128, 4 * HT * 512], BF16,
                         kind="ExternalInput")
    h1loc = nc.dram_tensor("h1loc", [128, HT * TOK], BF16,
                           kind="ExternalInput")
    W = []
    for l in range(L):
        W.append({
            "wq": nc.dram_tensor(f"wq{l}", [128, HT * H], BF16,
                                 kind="ExternalInput"),
            "wkv": nc.dram_tensor(f"wkv{l}", [128, HT * 128], BF16,
                                  kind="ExternalInput"),
            "wo": nc.dram_tensor(f"wo{l}", [128, HT * H], BF16,
                                 kind="ExternalInput"),
            "wg": nc.dram_tensor(f"wg{l}", [128, FT * H], BF16,
                                 kind="ExternalInput"),
            "wu": nc.dram_tensor(f"wu{l}", [128, FT * H], BF16,
                                 kind="ExternalInput"),
            "wd": nc.dram_tensor(f"wd{l}", [128, FT * H], BF16,
                                 kind="ExternalInput"),
        })
    xout = nc.dram_tensor("xout", [128, HT * TOK], F32, kind="ExternalOutput")
    dbg = {}
    if DEBUG:
        dbg["h1"] = nc.dram_tensor("dbg_h1", [128, HT * TOK], BF16,
                                   kind="ExternalOutput")
        dbg["qraw"] = nc.dram_tensor("dbg_qraw", [64, HQ * TOK], BF16,
                                     kind="ExternalOutput")
        dbg["kvb"] = nc.dram_tensor("dbg_kvb", [128, TOK], BF16,
                                    kind="ExternalOutput")
        dbg["kT"] = nc.dram_tensor("dbg_kT", [64, S], BF16,
                                   kind="ExternalOutput")
        dbg["vtok"] = nc.dram_tensor("dbg_vtok", [128, KT * 65], BF16,
                                     kind="ExternalOutput")
        dbg["q2"] = nc.dram_tensor("dbg_q2", [64, 512], BF16,
                                   kind="ExternalOutput")
        dbg["oT"] = nc.dram_tensor("dbg_oT", [128, HP * TOK], BF16,
                                   kind="ExternalOutput")
        dbg["xatt"] = nc.dram_tensor("dbg_xatt", [128, HT * TOK], F32,
                                     kind="ExternalOutput")
        dbg["act"] = nc.dram_tensor("dbg_act", [128, FT * TOK], BF16,
                                    kind="ExternalOutput")
        dbg["xl0"] = nc.dram_tensor("dbg_xl0", [128, HT * TOK], F32,
                                    kind="ExternalOutput")

    with tile.TileContext(nc) as tc:
        with (
            tc.tile_pool(name="const", bufs=1) as pconst,
            tc.tile_pool(name="resid", bufs=1) as presid,
            tc.tile_pool(name="hbuf", bufs=1) as ph,
            tc.tile_pool(name="qraw", bufs=1) as pq,
            tc.tile_pool(name="kvt", bufs=1) as pkv,
            tc.tile_pool(name="obuf", bufs=1) as poT,
            tc.tile_pool(name="act", bufs=1) as pact,
            tc.tile_pool(name="wbig", bufs=1) as pw,
            tc.tile_pool(name="wstream", bufs=6) as pws,
            tc.tile_pool(name="small", bufs=3) as psmall,
            tc.tile_pool(name="exp", bufs=3) as pexp,
            tc.tile_pool(name="x2p", bufs=1) as px2,
            tc.tile_pool(name="dram", bufs=2, space="DRAM") as pdram,
        ):
            # ---------------- constants ----------------
            identf = pconst.tile([128, 128], F32, tag="identf")
            make_identity(nc, identf[:])
            ident = pconst.tile([128, 128], BF16, tag="ident")
            nc.vector.tensor_copy(ident[:], identf[:])
            onesf = pconst.tile([128, 128], F32, tag="onesf")
            nc.vector.memset(onesf[:], 1.0)
            onesr = pconst.tile([128, 128], F32R, tag="onesr")
            nc.vector.tensor_copy(onesr[:], onesf[:])
            epst = pconst.tile([128, 1], F32, tag="eps")
            nc.gpsimd.memset(epst[:], EPS)
            cosq_sb = pconst.tile([64, 512], BF16, tag="cosq")
            sinq_sb = pconst.tile([64, 512], BF16, tag="sinq")
            nc.sync.dma_start(cosq_sb[:], cosq[:])
            nc.sync.dma_start(sinq_sb[:], sinq[:])
            cosa_sb = pconst.tile([64, S], BF16, tag="cosa")
            sina_sb = pconst.tile([64, S], BF16, tag="sina")
            nc.sync.dma_start(cosa_sb[:], cosa[:])
            nc.sync.dma_start(sina_sb[:], sina[:])
            h1l_sb = pconst.tile([128, HT * TOK], BF16, tag="h1l")
            nc.sync.dma_start(h1l_sb[:], h1loc[:])

            # warmup: absorb the collective setup / barrier cost early
            wrm_i = pdram.tile([128, 16], BF16, tag="warm_i")
            wrm_o = pdram.tile([NCORES * 128, 16], BF16, tag="warm_o",
                               addr_space="Shared")
            nc.gpsimd.collective_compute(
                "AllGather", mybir.AluOpType.bypass, replica_groups=RG,
                ins=[wrm_i[:].opt()], outs=[wrm_o[:].opt()],
            )

            # residual x, feature-major: block ht -> cols [ht*TOK:(ht+1)*TOK]
            # (loaded after the layer-0 critical-path DMAs, below)
            x_sb = presid.tile([128, HT * TOK], F32, tag="x")

            def rmsnorm(tag, dt=BF16):
                """x_sb -> h (same layout).  rsqrt via exp(-.5*ln(ms))
                to stay on the Exp/Ln activation table."""
                h_sb = ph.tile([128, HT * TOK], dt, tag="h" + str(dt))
                x2 = px2.tile([128, HT * TOK], F32R, tag="x2")
                with tc.tile_pool(name=f"ps_n_{tag}", bufs=1,
                                  space="PSUM") as pps:
                    ssq = pps.tile([1, TOK], F32, tag="ssq")
                    for ht in range(HT):
                        hs = slice(ht * TOK, (ht + 1) * TOK)
                        nc.vector.tensor_tensor(x2[:, hs], x_sb[:, hs],
                                                x_sb[:, hs], op=MUL)
                        nc.tensor.matmul(
                            ssq[:], onesr[:, 0:1], x2[:, hs],
                            start=(ht == 0), stop=(ht == HT - 1),
                        )
                    lnv = psmall.tile([1, TOK], F32, tag="lnv")
                    nc.scalar.activation(lnv[:], ssq[:], AF.Ln,
                                         bias=epst[0:1, :], scale=1.0 / H)
                    rinv = psmall.tile([1, TOK], F32R, tag="rinv")
                    nc.scalar.activation(rinv[:], lnv[:], AF.Exp, scale=-0.5)
                    rb = pps.tile([128, TOK], F32, tag="rb")
                    nc.tensor.matmul(rb[:], onesr[0:1, :], rinv[:],
                                     start=True, stop=True)
                    for ht in range(HT):
                        hs = slice(ht * TOK, (ht + 1) * TOK)
                        nc.vector.tensor_tensor(h_sb[:, hs], x_sb[:, hs],
                                                rb[:], op=MUL)
                return h_sb

            for l in range(L):
                w = W[l]
                # ---------------- weight loads ----------------
                # layer 0: kv-path DMAs first (critical path at startup);
                # wq/wo/x0 follow in the queue.  later layers: all up front.
                wq_sb = pw.tile([128, HT * H], BF16, tag="wq")
                wkv_sb = pw.tile([128, HT * 128], BF16, tag="wkv")
                wo_sb = pw.tile([128, HT * H], BF16, tag="wo")
                nc.sync.dma_start(wkv_sb[:], w["wkv"][:])
                if l > 0:
                    nc.sync.dma_start(wq_sb[:], w["wq"][:])
                    nc.sync.dma_start(wo_sb[:], w["wo"][:])

                if l == 0:
                    h1 = h1l_sb
                else:
                    h1 = rmsnorm(f"a{l}")

                with tc.tile_pool(name=f"ps_qkv{l}", bufs=1,
                                  space="PSUM") as pps:
                    kT = pkv.tile([128, S], BF16, tag="kT")
                    nc.vector.memset(kT[64:128, :], 0.0)
                    vT = pkv.tile([64, S], BF16, tag="vT")
                    if l == 0:
                        # layer 0: host-normalized h1 for ALL tokens is an
                        # input; compute k,v for all 2048 tokens locally
                        # (redundant per core) -- no collective needed, so
                        # the CC barrier overlaps real work.
                        for c4 in range(4):
                            h1c = pws.tile([128, HT * 512], BF16, tag="h1c",
                                           bufs=2)
                            nc.sync.dma_start(
                                h1c[:], h1a[:, c4 * HT * 512:
                                            (c4 + 1) * HT * 512])
                            pkv_ps = pps.tile([128, 512], F32, tag="pkv",
                                              bufs=2)
                            for k in range(HT):
                                nc.tensor.matmul(
                                    pkv_ps[:],
                                    wkv_sb[:, k * 128:(k + 1) * 128],
                                    h1c[:, k * 512:(k + 1) * 512],
                                    start=(k == 0), stop=(k == HT - 1),
                                )
                            cs4 = slice(c4 * 512, (c4 + 1) * 512)
                            rotk = psmall.tile([64, 512], BF16, tag="rotk")
                            nc.vector.tensor_copy(rotk[0:32, :],
                                                  pkv_ps[32:64, :])
                            nc.vector.tensor_copy(rotk[32:64, :],
                                                  pkv_ps[0:32, :])
                            nc.vector.tensor_tensor(
                                rotk[:], rotk[:], sina_sb[:, cs4], op=MUL)
                            nc.vector.tensor_tensor(
                                kT[0:64, cs4], pkv_ps[0:64, :],
                                cosa_sb[:, cs4], op=MUL)
                            nc.vector.tensor_tensor(
                                kT[0:64, cs4], kT[0:64, cs4], rotk[:],
                                op=ADD)
                            nc.vector.tensor_copy(vT[:, cs4],
                                                  pkv_ps[64:128, :])
                    else:
                        # local kv + rope + AllGather
                        kvb = psmall.tile([128, TOK], BF16, tag="kvb")
                        pkv_ps = pps.tile([128, 512], F32, tag="pkv",
                                          bufs=2)
                        for k in range(HT):
                            nc.tensor.matmul(
                                pkv_ps[:, 0:TOK],
                                wkv_sb[:, k * 128:(k + 1) * 128],
                                h1[:, k * TOK:(k + 1) * TOK],
                                start=(k == 0), stop=(k == HT - 1),
                            )
                        rotk = psmall.tile([64, 512], BF16, tag="rotk")
                        nc.vector.tensor_copy(rotk[0:32, 0:TOK],
                                              pkv_ps[32:64, 0:TOK])
                        nc.vector.tensor_copy(rotk[32:64, 0:TOK],
                                              pkv_ps[0:32, 0:TOK])
                        nc.vector.tensor_tensor(rotk[:, 0:TOK],
                                                rotk[:, 0:TOK],
                                                sinq_sb[:, 0:TOK], op=MUL)
                        nc.vector.tensor_tensor(kvb[0:64, :],
                                                pkv_ps[0:64, 0:TOK],
                                                cosq_sb[:, 0:TOK], op=MUL)
                        nc.vector.tensor_tensor(kvb[0:64, :], kvb[0:64, :],
                                                rotk[:, 0:TOK], op=ADD)
                        nc.vector.tensor_copy(kvb[64:128, :],
                                              pkv_ps[64:128, 0:TOK])

                        kv_in = pdram.tile([128, TOK], BF16, tag="kv_in")
                        nc.sync.dma_start(kv_in[:], kvb[:])
                        kv_out = pdram.tile([NCORES * 128, TOK], BF16,
                                            tag="kv_out",
                                            addr_space="Shared")
                        nc.gpsimd.collective_compute(
                            "AllGather", mybir.AluOpType.bypass,
                            replica_groups=RG,
                            ins=[kv_in[:].opt()], outs=[kv_out[:].opt()],
                        )

                    # ---------------- q projection ----------------
                    if l == 0:
                        nc.sync.dma_start(wq_sb[:], w["wq"][:])
                        nc.sync.dma_start(wo_sb[:], w["wo"][:])
                        nc.sync.dma_start(x_sb[:], x0[:])
                    # q_raw head-blocked [64, 16*256]: head h at cols h*TOK
                    q_raw = pq.tile([64, HQ * TOK], BF16, tag="q_raw")
                    for mp in range(4):      # m-tile pairs -> [128,512] psum
                        pq_ps = pps.tile([128, 2 * TOK], F32, tag="pq",
                                         bufs=3)
                        for j in range(2):
                            m = 2 * mp + j
                            for k in range(HT):
                                nc.tensor.matmul(
                                    pq_ps[:, j * TOK:(j + 1) * TOK],
                                    wq_sb[:, k * H + m * 128:
                                          k * H + (m + 1) * 128],
                                    h1[:, k * TOK:(k + 1) * TOK],
                                    start=(k == 0), stop=(k == HT - 1),
                                    skip_group_check=True,
                                )
                        for j in range(2):
                            m = 2 * mp + j
                            js = slice(j * TOK, (j + 1) * TOK)
                            nc.vector.tensor_copy(
                                q_raw[:, (2 * m) * TOK:(2 * m + 1) * TOK],
                                pq_ps[0:64, js])
                            nc.vector.tensor_copy(
                                q_raw[:, (2 * m + 1) * TOK:
                                      (2 * m + 2) * TOK],
                                pq_ps[64:128, js])

                    # ---------------- gather k/v, build v_tok ----------
                    if l > 0:
                        for s in range(NCORES):
                            nc.sync.dma_start(
                                kT[0:64, s * TOK:(s + 1) * TOK],
                                kv_out[s * 128:s * 128 + 64, :])
                            nc.sync.dma_start(
                                vT[:, s * TOK:(s + 1) * TOK],
                                kv_out[s * 128 + 64:(s + 1) * 128, :])
                    v_tok = pkv.tile([128, KT * 65], BF16, tag="v_tok")
                    nc.vector.memset(v_tok[:], 1.0)
                    for kt in range(KT):
                        pvt = pps.tile([128, 64], BF16, tag="pvt", bufs=2)
                        nc.tensor.transpose(
                            pvt[:], vT[:, kt * 128:(kt + 1) * 128],
                            ident[0:64, 0:64])
                        nc.vector.tensor_copy(
                            v_tok[:, kt * 65:kt * 65 + 64], pvt[:])

                if DEBUG and l == 0:
                    nc.sync.dma_start(dbg["qraw"][:], q_raw[:])
                    nc.sync.dma_start(dbg["kT"][:], kT[0:64, :])
                    nc.sync.dma_start(dbg["vtok"][:], v_tok[:])
                # ---------------- rope q (all head pairs up front) --------
                q2s = []
                for hp in range(HP):
                    cs = slice(hp * 512, (hp + 1) * 512)
                    q2 = psmall.tile([128, 512], BF16, tag="q2", bufs=8)
                    nc.vector.memset(q2[64:128, :], 0.0)
                    rot = psmall.tile([64, 512], BF16, tag="rotq", bufs=2)
                    nc.vector.tensor_copy(rot[0:32, :], q_raw[32:64, cs])
                    nc.vector.tensor_copy(rot[32:64, :], q_raw[0:32, cs])
                    nc.vector.tensor_tensor(rot[:], rot[:], sinq_sb[:],
                                            op=MUL)
                    nc.vector.tensor_tensor(q2[0:64, :], q_raw[:, cs],
                                            cosq_sb[:], op=MUL)
                    nc.vector.tensor_tensor(q2[0:64, :], q2[0:64, :],
                                            rot[:], op=ADD)
                    q2s.append(q2)
                if DEBUG and l == 0:
                    nc.sync.dma_start(dbg["q2"][:], q2s[0][0:64, :])

                # ---------------- attention ----------------
                # oT [128, 8*256]: block hp holds heads 2hp (rows 0:64) and
                # 2hp+1 (rows 64:128) for the core's 256 tokens.
                oT = poT.tile([128, HP * TOK], BF16, tag="oT")
                with tc.tile_pool(name=f"ps_att{l}", bufs=1,
                                  space="PSUM") as ppa:
                    # Software-pipelined: av runs 2 ktp slots behind scores
                    # (exp always complete -> PE never stalls mid-pair), and
                    # each pair's finalize (recip/broadcast/normalize) is
                    # interleaved into the NEXT pair's score stream so the
                    # PE never head-of-line blocks at a pair boundary.
                    pavs = {}
                    pend = []       # [(hp, ktp, et, last)] awaiting av
                    fin_pend = []   # [(hp, rec)] awaiting broadcast+norm

                    def emit_av(hp, ktp, et, last):
                        pav = pavs[hp]
                        for j in range(2):
                            kt = 2 * ktp + j
                            nc.tensor.matmul(
                                pav[:], v_tok[:, kt * 65:(kt + 1) * 65],
                                et[:, j * 512:(j + 1) * 512],
                                start=(ktp == 0 and j == 0),
                                stop=(last and j == 1),
                                skip_group_check=True,
                            )
                        if last:
                            rec = psmall.tile([1, 512], F32R, tag="rec",
                                              bufs=2)
                            with nc.allow_low_precision(reason="f32r"):
                                nc.vector.reciprocal(rec[:], pav[64:65, :])
                            fin_pend.append((hp, rec))

                    def emit_fin(hp, rec):
                        pav = pavs.pop(hp)
                        prb = ppa.tile([128, 1024], F32, tag="psc", bufs=3)
                        nc.tensor.matmul(prb[0:64, 0:512],
                                         onesr[0:1, 0:64], rec[:],
                                         start=True, stop=True)
                        rbs = psmall.tile([64, 512], F32, tag="rbs", bufs=2)
                        nc.vector.tensor_copy(rbs[:], prb[0:64, 0:512])
                        nc.vector.tensor_tensor(
                            oT[0:64, hp * TOK:(hp + 1) * TOK],
                            pav[0:64, 0:TOK], rbs[:, 0:TOK], op=MUL)
                        nc.vector.tensor_tensor(
                            oT[64:128, hp * TOK:(hp + 1) * TOK],
                            pav[0:64, TOK:512], rbs[:, TOK:512], op=MUL)

                    for hp in range(HP):
                        q2 = q2s[hp]
                        pav = ppa.tile([65, 512], F32, tag="pav", bufs=2)
                        pavs[hp] = pav
                        for ktp in range(8):
                            psc = ppa.tile([128, 1024], F32, tag="psc",
                                           bufs=3)
                            for j in range(2):
                                kt = 2 * ktp + j
                                nc.tensor.matmul(
                                    psc[:, j * 512:(j + 1) * 512],
                                    kT[:, kt * 128:(kt + 1) * 128],
                                    q2[:], start=True, stop=True,
                                    skip_group_check=True,
                                )
                            et = pexp.tile([128, 1024], BF16, tag="et",
                                           bufs=5)
                            nc.scalar.activation(et[:], psc[:], AF.Exp)
                            if len(pend) >= 3:
                                emit_av(*pend.pop(0))
                            if fin_pend and ktp == 4:
                                emit_fin(*fin_pend.pop(0))
                            pend.append((hp, ktp, et, ktp == 7))
                    for job in pend:
                        emit_av(*job)
                    for hp_f, rec_f in fin_pend:
                        emit_fin(hp_f, rec_f)

                # ---------------- o-proj + residual ----------------
                with tc.tile_pool(name=f"ps_o{l}", bufs=1,
                                  space="PSUM") as ppo:
                    for mp in range(4):
                        po = ppo.tile([128, 2 * TOK], F32, tag="po", bufs=2)
                        for j in range(2):
                            m = 2 * mp + j
                            for hp in range(HP):
                                nc.tensor.matmul(
                                    po[:, j * TOK:(j + 1) * TOK],
                                    wo_sb[:, hp * H + m * 128:
                                          hp * H + (m + 1) * 128],
                                    oT[:, hp * TOK:(hp + 1) * TOK],
                                    start=(hp == 0), stop=(hp == HT - 1),
                                    skip_group_check=True,
                                )
                        for j in range(2):
                            m = 2 * mp + j
                            ms = slice(m * TOK, (m + 1) * TOK)
                            nc.vector.tensor_tensor(
                                x_sb[:, ms], x_sb[:, ms],
                                po[:, j * TOK:(j + 1) * TOK], op=ADD)

                if DEBUG and l == 0:
                    nc.sync.dma_start(dbg["oT"][:], oT[:])
                    nc.sync.dma_start(dbg["xatt"][:], x_sb[:])
                # ---------------- mlp ----------------
                h2 = rmsnorm(f"m{l}")
                act = pact.tile([128, FT * TOK], BF16, tag="act")
                with tc.tile_pool(name=f"ps_mlp{l}", bufs=1,
                                  space="PSUM") as ppm:
                    for f in range(FT):
                        wgf = pws.tile([128, H], BF16, tag="wgf")
                        wuf = pws.tile([128, H], BF16, tag="wuf")
                        nc.sync.dma_start(wgf[:],
                                          w["wg"][:, f * H:(f + 1) * H])
                        nc.sync.dma_start(wuf[:],
                                            w["wu"][:, f * H:(f + 1) * H])
                        # NOTE: start=True marks the whole 2KB PSUM bank
                        # pending-zero, so the g and u accumulation groups
                        # sharing this tile must run sequentially, not
                        # interleaved.
                        pgu = ppm.tile([128, 2 * TOK], F32, tag="pgu",
                                       bufs=3)
                        for k in range(HT):
                            nc.tensor.matmul(
                                pgu[:, 0:TOK],
                                wgf[:, k * 128:(k + 1) * 128],
                                h2[:, k * TOK:(k + 1) * TOK],
                                start=(k == 0), stop=(k == HT - 1),
                                skip_group_check=True,
                            )
                        for k in range(HT):
                            nc.tensor.matmul(
                                pgu[:, TOK:2 * TOK],
                                wuf[:, k * 128:(k + 1) * 128],
                                h2[:, k * TOK:(k + 1) * TOK],
                                start=(k == 0), stop=(k == HT - 1),
                                skip_group_check=True,
                            )
                        fs = slice(f * TOK, (f + 1) * TOK)
                        asl = psmall.tile([128, TOK], BF16, tag="asl")
                        nc.scalar.activation(asl[:], pgu[:, 0:TOK], AF.Silu)
                        nc.vector.tensor_tensor(act[:, fs], asl[:],
                                                pgu[:, TOK:2 * TOK], op=MUL)

                    # down: m outer (one sequential accumulation group per
                    # psum tile), fk inner; wd streamed in per-m 1MB chunks
                    for m in range(HT):
                        wdm = pws.tile([128, FT * 128], BF16, tag="wdm",
                                       bufs=2)
                        nc.sync.dma_start(
                            wdm[:], w["wd"][:, m * FT * 128:
                                            (m + 1) * FT * 128])
                        pd = ppm.tile([128, TOK], F32, tag="pd", bufs=3)
                        for fk in range(FT):
                            nc.tensor.matmul(
                                pd[:], wdm[:, fk * 128:(fk + 1) * 128],
                                act[:, fk * TOK:(fk + 1) * TOK],
                                start=(fk == 0), stop=(fk == FT - 1),
                            )
                        ms = slice(m * TOK, (m + 1) * TOK)
                        nc.vector.tensor_tensor(
                            x_sb[:, ms], x_sb[:, ms], pd[:], op=ADD)
                        if l == L - 1:
                            nc.sync.dma_start(xout[:, ms],
                                              x_sb[:, ms])
                if DEBUG and l == 0:
                    nc.sync.dma_start(dbg["act"][:], act[:])
                    nc.sync.dma_start(dbg["xl0"][:], x_sb[:])

    return nc


def _get_nc():
    global _CACHED_NC
    if _CACHED_NC is None:
        _CACHED_NC = _build_nc()
    return _CACHED_NC


def _pack_km(wT, K, M):
    """[K*128, M*128] (contract-major) -> [128, K*M*128], col k*M*128+m*128+c
    = tile (k, m) so tile slice [:, k*(M*128)+m*128 : +128] is lhsT."""
    return np.ascontiguousarray(
        wT.reshape(K, 128, M, 128).transpose(1, 0, 2, 3).reshape(128, -1))


def _pack_fk(wT, K, Fn):
    """[K*128, Fn*128] -> [128, Fn*K*128], col f*(K*128)+k*128+c = tile
    (k, f): f-chunk [:, f*K*128 : (f+1)*K*128] holds all K contract tiles."""
    return np.ascontiguousarray(
        wT.reshape(K, 128, Fn, 128).transpose(1, 2, 0, 3).reshape(128, -1))


def _host_prep(inputs):
    """Fold ln/scale into weights, pre-pack DMA-friendly layouts, embed
    gather, per-core rope tables.  Returns in_maps (list of dicts)."""
    ids = np.asarray(inputs["input_ids"])[0]          # [S] int32
    embed = np.asarray(inputs["embed"], np.float32)   # [V, H]
    x = embed[ids]                                    # [S, H]
    ms = (x * x).mean(-1, keepdims=True) + EPS
    h1_full = (x / np.sqrt(ms)).astype(np.float32)    # pre-ln-fold norm

    inv = 1.0 / (10000.0 ** (np.arange(0, D, 2, dtype=np.float32) / D))
    scale = np.float32(1.0 / np.sqrt(D))

    def bf(a):
        return np.ascontiguousarray(a).astype(BF)

    # shared (per-layer) weights, packed once
    shared = {}
    for l in range(L):
        ln1 = np.asarray(inputs["ln1"], np.float32)[l]
        ln2 = np.asarray(inputs["ln2"], np.float32)[l]
        wq = np.asarray(inputs["Wq"], np.float32)[l] * ln1[None, :] * scale
        wk = np.asarray(inputs["Wk"], np.float32)[l] * ln1[None, :]
        wv = np.asarray(inputs["Wv"], np.float32)[l] * ln1[None, :]
        wo = np.asarray(inputs["Wo"], np.float32)[l]
        wg = np.asarray(inputs["Wg"], np.float32)[l] * ln2[None, :]
        wu = np.asarray(inputs["Wu"], np.float32)[l] * ln2[None, :]
        wd = np.asarray(inputs["Wd"], np.float32)[l]
        wkv = np.concatenate([wk, wv], axis=0)        # [128, H]
        shared[f"wq{l}"] = bf(_pack_km(wq.T, HT, HT))
        shared[f"wkv{l}"] = bf(_pack_km(wkv.T, HT, 1))
        shared[f"wo{l}"] = bf(_pack_km(wo.T.reshape(H, H), HT, HT))
        shared[f"wg{l}"] = bf(_pack_fk(wg.T, HT, FT))
        shared[f"wu{l}"] = bf(_pack_fk(wu.T, HT, FT))
        shared[f"wd{l}"] = bf(_pack_fk(wd.T, FT, HT))

    # global rope tables + packed h1_all (shared across cores)
    frA = np.arange(S, dtype=np.float32)[:, None] * inv[None, :]
    cosTA = np.cos(frA).T.astype(np.float32)
    sinTA = np.sin(frA).T.astype(np.float32)
    cosA = np.tile(cosTA, (2, 1))                     # [64, S]
    sinA = np.concatenate([-sinTA, sinTA], 0)
    h1a = np.ascontiguousarray(
        h1_full.T.reshape(HT, 128, 4, 512).transpose(1, 2, 0, 3)
        .reshape(128, -1))
    shared["cosa"] = bf(cosA)
    shared["sina"] = bf(sinA)
    shared["h1a"] = bf(h1a)

    in_maps = []
    for c in range(NCORES):
        pos = np.arange(c * TOK, (c + 1) * TOK, dtype=np.float32)
        freqs = pos[:, None] * inv[None, :]           # [TOK, 32]
        cosT = np.cos(freqs).T.astype(np.float32)     # [32, TOK]
        sinT = np.sin(freqs).T.astype(np.float32)
        cosQ = np.tile(np.tile(cosT, (2, 1)), (1, 2))             # [64, 512]
        sinQ = np.tile(np.concatenate([-sinT, sinT], 0), (1, 2))  # [64, 512]
        xT = x[c * TOK:(c + 1) * TOK, :].T            # [H, TOK]
        x0p = np.ascontiguousarray(
            xT.reshape(HT, 128, TOK).transpose(1, 0, 2).reshape(128, -1))
        h1loc = np.ascontiguousarray(
            h1_full[c * TOK:(c + 1) * TOK].T.reshape(HT, 128, TOK)
            .transpose(1, 0, 2).reshape(128, -1))
        m = {"x0": x0p, "cosq": bf(cosQ), "sinq": bf(sinQ),
             "h1loc": bf(h1loc)}
        m.update(shared)
        in_maps.append(m)
    return in_maps


def kernel(**inputs) -> np.ndarray:
    nc = _get_nc()
    in_maps = _host_prep(inputs)
    res = bass_utils.run_bass_kernel_spmd(
        nc, in_maps, core_ids=list(range(NCORES))
    )
    out = np.empty((1, S, H), np.float32)
    for c in range(NCORES):
        xp = res.results[c]["xout"]                   # [128, HT*TOK]
        xT = xp.reshape(128, HT, TOK).transpose(1, 0, 2).reshape(H, TOK)
        out[0, c * TOK:(c + 1) * TOK, :] = xT.T
    return out
